# revision 7
# baseline (speedup 1.0000x reference)
"""Trainium2 Bass kernel: SNN Leaky-Integrate-and-Fire layer.

Computes, for x [T=1024, N_IN=4096] f32 and W [N_OUT=4096, N_IN=4096] f32:
    cur = x @ W.T                                   # [T, N_OUT]
    mem_t = 0.9*mem_{t-1} + cur_t - (mem_{t-1} > 1)  # scan over T
    spk_t = (mem_t > 1)
returning (spk_rec, mem_rec), both [T, N_OUT] f32.

Sharding: N_OUT split across 8 NeuronCores (512 neurons each). x is
replicated (each core reads the T-half slices it needs exactly once).

Device algorithm per core (neurons on partitions, time along free dim):
  cur[o, t] accumulated in PSUM via 256 matmuls (K=4096 in 32 tiles,
  O=512 in 4 tiles of 128 partitions, T=1024 in 2 half-banks of 512).
  The scan is decomposed as mem = A + B with
     A_t = 0.9*A_{t-1} + cur_t        (pure linear -> one HW
                                       tensor_tensor_scan per 512 steps)
     B_t = 0.9*B_{t-1} - s_{t-1}      (spike-coupled residual)
     s_t = (B_t > theta_t),  theta_t = 1 - A_t   (precomputed in bulk)
  so the serial part is only 2 small DVE instructions per timestep.
"""

import numpy as np

T = 1024
N_IN = 4096
N_OUT = 4096
N_CORES = 8
O_SHARD = N_OUT // N_CORES  # 512
KT = N_IN // 128  # 32 k-tiles
OT = O_SHARD // 128  # 4 o-tiles
BETA = 0.9
THRESHOLD = 1.0

_CACHE = {}


def _build_nc(mm_dtype_name: str):
    import concourse.bacc as bacc
    import concourse.mybir as mybir
    from concourse.tile import TileContext

    F32 = mybir.dt.float32
    MMDT = getattr(mybir.dt, mm_dtype_name)
    Op = mybir.AluOpType

    nc = bacc.Bacc(target_bir_lowering=False)
    xT_d = nc.dram_tensor("xT", [N_IN, T], MMDT, kind="ExternalInput")
    WT_d = nc.dram_tensor("WT", [N_IN, O_SHARD], MMDT, kind="ExternalInput")
    spk_d = nc.dram_tensor("spk", [O_SHARD, T], F32, kind="ExternalOutput")
    mem_d = nc.dram_tensor("mem", [O_SHARD, T], F32, kind="ExternalOutput")

    with TileContext(nc) as tc:
        with (
            tc.tile_pool(name="sb", bufs=1) as sb,
            tc.tile_pool(name="xs", bufs=4) as xs,
            tc.tile_pool(name="psp", bufs=1, space="PSUM") as psp,
        ):
            # All weights resident: [128, KT, O_SHARD]; k-tile k holds
            # WT rows k*128..k*128+127 (i.e. W.T), so wt[:, k, o*128:...]
            # is directly the matmul stationary operand [K=128, M=128].
            wt = sb.tile([128, KT, O_SHARD], MMDT, name="wt")
            wt_view = WT_d.rearrange("(k p) o -> p k o", p=128)
            for kc in range(0, KT, 4):
                nc.sync.dma_start(wt[:, kc : kc + 4, :], wt_view[:, kc : kc + 4, :])

            ps = [
                psp.tile([128, T], F32, name=f"ps{o}", tag=f"ps{o}") for o in range(OT)
            ]

            A = sb.tile([128, OT, T], F32, name="A")  # linear-part scan
            TH = sb.tile([128, OT, T], F32, name="TH")  # theta = 1 - A
            M = sb.tile([128, OT, T], F32, name="M")  # mem = A + B
            Bb = sb.tile([128, OT, T + 1], F32, name="Bb")  # residual state
            Sb = sb.tile([128, OT, T + 1], F32, name="Sb")  # spikes (0/1)
            beta_t = sb.tile([128, 512], F32, name="beta_t")

            nc.vector.memset(beta_t, BETA)
            nc.vector.memset(Bb[:, :, 0], 0.0)
            nc.vector.memset(Sb[:, :, 0], 0.0)

            for th in range(2):
                tl, tr = th * 512, (th + 1) * 512
                # ---- matmul: accumulate cur[:, tl:tr] over all K ----
                for k in range(KT):
                    xh = xs.tile([128, 512], MMDT, name="xh")
                    nc.sync.dma_start(xh, xT_d[k * 128 : (k + 1) * 128, tl:tr])
                    for o in range(OT):
                        nc.tensor.matmul(
                            ps[o][:, tl:tr],
                            lhsT=wt[:, k, o * 128 : (o + 1) * 128],
                            rhs=xh,
                            start=(k == 0),
                            stop=(k == KT - 1),
                        )
                # ---- bulk prep for this half: A scan + theta ----
                for o in range(OT):
                    init = 0.0 if th == 0 else A[:, o, tl - 1 : tl]
                    nc.vector.tensor_tensor_scan(
                        out=A[:, o, tl:tr],
                        data0=beta_t,
                        data1=ps[o][:, tl:tr],
                        initial=init,
                        op0=Op.mult,
                        op1=Op.add,
                    )
                    nc.gpsimd.tensor_scalar(
                        TH[:, o, tl:tr], A[:, o, tl:tr], -1.0, THRESHOLD, Op.mult, Op.add
                    )
                # ---- serial scan for this half: 2 DVE instrs per step ----
                for t in range(tl + 1, tr + 1):
                    nc.vector.scalar_tensor_tensor(
                        out=Bb[:, :, t],
                        in0=Bb[:, :, t - 1],
                        scalar=BETA,
                        in1=Sb[:, :, t - 1],
                        op0=Op.mult,
                        op1=Op.subtract,
                    )
                    nc.vector.tensor_tensor(
                        Sb[:, :, t], Bb[:, :, t], TH[:, :, t - 1], Op.is_gt
                    )
                # ---- epilogue for this half: mem = A + B, DMA out ----
                for o in range(OT):
                    nc.gpsimd.tensor_tensor(
                        M[:, o, tl:tr], A[:, o, tl:tr], Bb[:, o, tl + 1 : tr + 1], Op.add
                    )
                    nc.sync.dma_start(
                        spk_d[o * 128 : (o + 1) * 128, tl:tr], Sb[:, o, tl + 1 : tr + 1]
                    )
                    nc.sync.dma_start(
                        mem_d[o * 128 : (o + 1) * 128, tl:tr], M[:, o, tl:tr]
                    )
    nc.finalize()
    return nc


def _get_nc(mm_dtype_name: str):
    if mm_dtype_name not in _CACHE:
        _CACHE[mm_dtype_name] = _build_nc(mm_dtype_name)
    return _CACHE[mm_dtype_name]


def run(x, W, mm_dtype_name="float32r", trace=False):
    from concourse.bass_utils import run_bass_kernel_spmd

    nc = _get_nc(mm_dtype_name)
    xT = np.ascontiguousarray(np.asarray(x, dtype=np.float32).T)  # [N_IN, T]
    W = np.asarray(W, dtype=np.float32)
    in_maps = []
    for c in range(N_CORES):
        WTc = np.ascontiguousarray(W[c * O_SHARD : (c + 1) * O_SHARD, :].T)
        in_maps.append({"xT": xT, "WT": WTc})
    res = run_bass_kernel_spmd(nc, in_maps, core_ids=list(range(N_CORES)), trace=trace)
    spk = np.concatenate([r["spk"] for r in res.results], axis=0).T
    mem = np.concatenate([r["mem"] for r in res.results], axis=0).T
    return (
        np.ascontiguousarray(spk),
        np.ascontiguousarray(mem),
    ), res


def kernel(x, W):
    out, _ = run(x, W)
    return out
